# revision 43
# baseline (speedup 1.0000x reference)
"""Trainium2 Bass kernel: BiLSTM dependency-parser edge scorer.

Self-contained. Accepts FULL inputs (as produced by setup_inputs()), returns
the FULL [65025, 1] float32 score tensor.

Strategy (per NeuronCore, SPMD over 8 cores; replicated except the edge-score
row selection):
  - The LSTM recurrences are solved by Jacobi fixed-point iteration over the
    time-unrolled network: sweep k computes gates = xg + Whh @ H^(k-1) for ALL
    256 timesteps as batched matmuls (h-feedback lagged one sweep), applies
    sigmoid/tanh as wide activation ops, runs the c-recurrence with the DVE
    tensor_tensor_scan instruction, and rebuilds h = sigmoid(o) * tanh(c).
  - Intermediate sweeps run entirely in a x256-scaled fp8-e4m3 domain:
    recurrent matmuls in DoubleRow mode (two 100-row k-subtiles per
    instruction at 0.5 cyc/row), input projections xg pre-computed by a
    DoubleRow GEMM and stored as fp8 (x16); a (16*I, 0) DoubleRow identity
    copy injects them into the gate psum. tanh(c) is skipped (|c| < 0.45, so
    th := c). These approximations shift the intermediate fixed point by a
    contraction-healable amount only.
  - The final sweeps per layer run with exact fp16 xg (x256) and fp16
    recurrent matmuls, converging onto the true fixed point. The exact-xg
    GEMMs are emitted as PE filler chunks BETWEEN the intermediate sweeps'
    matmul bursts, so they stay off the critical path (and keep the PE
    p-state hot). Activations un-scale via the ACT scale operand (exact).
  - Gate layout: 16 tiles of 100 rows, tile = 4*gate_group + j, gate-group
    order (i, g, f, o); the gate psum is 4 quarter tiles per direction (one
    gate group each, 2 PSUM banks each), leaving 2 banks for filler GEMMs.
  - H is stored transposed ([100 hidden, 4 j-blocks, 258] with zero guard
    columns) in fp8 (x16, feeding DoubleRow sweeps) and fp16 (unscaled,
    feeding the fp16 sweeps, the layer-1 projections, and the edge GEMMs).
  - Edge MLP: scores[h,m] = w2 . tanh(A[h] + B[m] + b1) + b2 with
    A = h1 @ Uh^T, B = h1 @ Um^T. Each core computes a [32, 256] slice of the
    score grid (rows picked by a per-core one-hot input); host assembles.
"""

import os
import sys

sys.path.insert(0, "/opt/trn_rl_repo")

import numpy as np

import concourse.bass as bass
import concourse.mybir as mybir
from concourse import bacc
from concourse.bass import IndirectOffsetOnAxis
from concourse.masks import make_identity
from concourse.tile import TileContext

N = 256          # sequence length
NC = 8           # cores
F32 = mybir.dt.float32
BF16 = mybir.dt.float16
FP8 = mybir.dt.float8e4
I32 = mybir.dt.int32
AF = mybir.ActivationFunctionType
OP = mybir.AluOpType
DR = mybir.MatmulPerfMode.DoubleRow

# per-layer sweep schedule (DP_MODES, same string for both layers):
#   'x' = no recurrent matmul (gates = fp8 xg), tanh(c) skipped (th := c)
#   's' = fp8 DoubleRow recurrent matmul + fp8 xg, tanh(c) skipped
#   't' = fp8 DoubleRow recurrent matmul + fp8 xg, real tanh(c)
#   '6' = fp16 recurrent matmul + exact fp16 xg, real tanh(c)
MODES = os.environ.get("DP_MODES", "xssss6666")

SCALE = 256.0          # gate-psum scale
INV_SCALE = 1.0 / SCALE

# tile-group order (i, g, f, o)
_OG = (0, 2, 1, 3)


# ---------------------------------------------------------------------------
# host-side weight layout prep
# ---------------------------------------------------------------------------


def _bf(a):
    return np.ascontiguousarray(np.asarray(a).astype(np.float16))


def _f8(a):
    import ml_dtypes
    return np.ascontiguousarray(np.asarray(a).astype(ml_dtypes.float8_e4m3))


def _rows(tt):
    """Original gate-row indices (torch order i,f,g,o) for tile tt."""
    return 400 * _OG[tt // 4] + 100 * (tt % 4) + np.arange(100)


def _whh_lay(W):
    """W [1600, 400] -> [100 k, 6400] with free = 400*tt + 100*j + m."""
    out = np.zeros((100, 6400), np.float64)
    for tt in range(16):
        R = np.asarray(W, np.float64)[_rows(tt)]      # [100 m, 400]
        for j in range(4):
            out[:, 400 * tt + 100 * j: 400 * tt + 100 * j + 100] = \
                R[:, 100 * j: 100 * j + 100].T
    return out


def _wih_lay(W, nch):
    """W [1600, 100*nch] -> [100 k, .]: free = (100*nch)*tt + 100*ch + m."""
    D = 100 * nch
    out = np.zeros((100, 16 * D), np.float64)
    for tt in range(16):
        R = np.asarray(W, np.float64)[_rows(tt)]      # [100 m, D]
        for ch in range(nch):
            out[:, D * tt + 100 * ch: D * tt + 100 * ch + 100] = \
                R[:, 100 * ch: 100 * ch + 100].T
    return out


def _bias_lay(b):
    """b [1600] -> [1600] with index 100*tt + m."""
    out = np.zeros(1600, np.float64)
    for tt in range(16):
        out[100 * tt: 100 * tt + 100] = np.asarray(b, np.float64)[_rows(tt)]
    return out


def _prep_inputs(word_idx, pos_idx, word_emb, pos_emb,
                 Wih0, Whh0, bih0, bhh0, Wih1, Whh1, bih1, bhh1,
                 fc1_W, fc1_b, fc2_W, fc2_b):
    arr = {}
    wi = np.asarray(word_idx).reshape(N).astype(np.int32)
    pi = np.asarray(pos_idx).reshape(N).astype(np.int32)
    arr["idx4"] = np.ascontiguousarray(
        np.stack([wi[:128], wi[128:], pi[:128], pi[128:]], axis=1))
    arr["wemb"] = np.ascontiguousarray(np.asarray(word_emb, dtype=np.float32))
    arr["pemb"] = np.ascontiguousarray(np.asarray(pos_emb, dtype=np.float32))

    whh = np.zeros((4, 100, 6400), np.float64)
    wih0 = np.zeros((2, 100, 6400), np.float64)
    wih1 = np.zeros((2, 100, 12800), np.float64)
    bias = np.zeros((2, 3200), np.float64)
    for d in range(2):
        whh[2 * 0 + d] = _whh_lay(np.asarray(Whh0)[d])
        whh[2 * 1 + d] = _whh_lay(np.asarray(Whh1)[d])
        wih0[d] = _wih_lay(np.asarray(Wih0)[d], 4)
        wih1[d] = _wih_lay(np.asarray(Wih1)[d], 8)
        bias[0, 1600 * d: 1600 * d + 1600] = _bias_lay(
            np.asarray(bih0)[d] + np.asarray(bhh0)[d])
        bias[1, 1600 * d: 1600 * d + 1600] = _bias_lay(
            np.asarray(bih1)[d] + np.asarray(bhh1)[d])
    # fp16 weights carry the full 256x psum scale (their rhs are unscaled)
    arr["whh"] = _bf(whh * SCALE)
    arr["wih0"] = _bf(wih0 * SCALE)
    arr["wih1"] = _bf(wih1 * SCALE)
    arr["bias0"] = _bf(bias[0:1] * SCALE)
    arr["bias1"] = _bf(bias[1:2] * SCALE)
    # fp8 weights carry 16x (their rhs carry the other 16x)
    # DoubleRow weights need a 32-aligned subtile stride: pad M 100 -> 128
    whh8p = np.zeros((4, 128, 16, 4, 128), np.float64)
    whh8p[:, 0:100, :, :, 0:100] = (whh * 16.0).reshape(4, 100, 16, 4, 100)
    arr["whh8"] = _f8(whh8p)
    w08 = np.zeros((2, 128, 16, 4, 128), np.float64)
    w08[:, 0:100, :, :, 0:100] = (wih0 * 16.0).reshape(2, 100, 16, 4, 100)
    w18 = np.zeros((2, 128, 16, 8, 128), np.float64)
    w18[:, 0:100, :, :, 0:100] = (wih1 * 16.0).reshape(2, 100, 16, 8, 100)
    for d in range(2):
        for tt in range(16):
            w08[d, 100, tt, 0, 0:100] = bias[0, 1600 * d + 100 * tt:
                                             1600 * d + 100 * tt + 100] * 16.0
            w18[d, 100, tt, 0, 0:100] = bias[1, 1600 * d + 100 * tt:
                                             1600 * d + 100 * tt + 100] * 16.0
    arr["wih08"] = _f8(w08)
    arr["wih18"] = _f8(w18)
    # DoubleRow identity pairs: rows (16I, 0, 0, 16I) -> (16I,0) and (0,16I)
    idn8 = np.zeros((128, 4, 128), np.float64)
    idn8[0:100, 0, 0:100] = 16.0 * np.eye(100)
    idn8[0:100, 3, 0:100] = 16.0 * np.eye(100)
    arr["idn8"] = _f8(idn8)

    # edge MLP: Uh = fc1_W[:, :800].T chunks, Um = fc1_W[:, 800:].T chunks
    f1 = np.asarray(fc1_W, np.float64)
    uh = np.zeros((100, 800), np.float64)
    um = np.zeros((100, 800), np.float64)
    for c in range(8):
        uh[:, 100 * c: 100 * c + 100] = f1[:, 100 * c: 100 * c + 100].T
        um[:, 100 * c: 100 * c + 100] = f1[:, 800 + 100 * c: 900 + 100 * c].T
    arr["uh"] = _bf(uh)
    arr["um"] = _bf(um)
    arr["w2"] = _bf(np.asarray(fc2_W, np.float64).reshape(100, 1))
    arr["b1"] = np.ascontiguousarray(
        np.asarray(fc1_b, np.float32).reshape(100, 1))
    arr["b2"] = np.ascontiguousarray(
        np.full((128, 1), np.float32(np.asarray(fc2_b).reshape(())),
                dtype=np.float32))
    return arr


def _make_selT(core):
    s = np.zeros((2, 128, 32), np.float32)
    for r in range(32):
        t = 32 * core + r
        s[t // 128, t % 128, r] = 1.0
    return np.ascontiguousarray(np.concatenate([s[0], s[1]], axis=1))


# ---------------------------------------------------------------------------
# device kernel build
# ---------------------------------------------------------------------------


def build_nc():
    nc = bacc.Bacc("TRN2", target_bir_lowering=False, debug=False,
                   num_devices=NC)
    wemb = nc.dram_tensor("wemb", [50000, 300], F32, kind="ExternalInput").ap()
    pemb = nc.dram_tensor("pemb", [50, 100], F32, kind="ExternalInput").ap()
    idx4d = nc.dram_tensor("idx4", [128, 4], I32, kind="ExternalInput").ap()
    whhd = nc.dram_tensor("whh", [4, 100, 6400], BF16, kind="ExternalInput").ap()
    whh8d = nc.dram_tensor("whh8", [4, 128, 16, 4, 128], FP8, kind="ExternalInput").ap()
    wih0d = nc.dram_tensor("wih0", [2, 100, 6400], BF16, kind="ExternalInput").ap()
    wih1d = nc.dram_tensor("wih1", [2, 100, 12800], BF16, kind="ExternalInput").ap()
    wih08d = nc.dram_tensor("wih08", [2, 128, 16, 4, 128], FP8, kind="ExternalInput").ap()
    wih18d = nc.dram_tensor("wih18", [2, 128, 16, 8, 128], FP8, kind="ExternalInput").ap()
    bias0d = nc.dram_tensor("bias0", [1, 3200], BF16, kind="ExternalInput").ap()
    bias1d = nc.dram_tensor("bias1", [1, 3200], BF16, kind="ExternalInput").ap()
    idn8d = nc.dram_tensor("idn8", [128, 4, 128], FP8, kind="ExternalInput").ap()
    uhd = nc.dram_tensor("uh", [100, 800], BF16, kind="ExternalInput").ap()
    umd = nc.dram_tensor("um", [100, 800], BF16, kind="ExternalInput").ap()
    w2d = nc.dram_tensor("w2", [100, 1], BF16, kind="ExternalInput").ap()
    b1d = nc.dram_tensor("b1", [100, 1], F32, kind="ExternalInput").ap()
    b2d = nc.dram_tensor("b2", [128, 1], F32, kind="ExternalInput").ap()
    selTd = nc.dram_tensor("selT", [128, 64], F32, kind="ExternalInput").ap()
    grid = nc.dram_tensor("grid", [32, N], F32, kind="ExternalOutput").ap()

    K = len(MODES)

    from contextlib import ExitStack
    with TileContext(nc) as tc, ExitStack() as ctx:
        top = ctx.enter_context(tc.tile_pool(name="top", bufs=1))
        # recurrent weights: 2 shared tiles per format, reloaded for layer 1
        whh_sb = [top.tile([100, 6400], BF16, name=f"whh{d}", tag=f"whh{d}")
                  for d in range(2)]
        whh8_sb = [top.tile([128, 16, 4, 128], FP8, name=f"wh8{d}", tag=f"wh8{d}")
                   for d in range(2)]
        bias_sb = [top.tile([1, 3200], BF16, name=f"bias{l}", tag=f"bias{l}")
                   for l in range(2)]
        idn8_sb = top.tile([128, 4, 128], FP8, name="idn8", tag="idn8")
        idn128 = top.tile([128, 128], F32, name="idn128", tag="idn128")
        make_identity(nc, idn128[:, :])
        ones_sb = top.tile([1, N], BF16, name="ones", tag="ones")
        nc.vector.memset(ones_sb[:, :], 1.0)
        # xg tiles, shared between layers: fp16 exact (x256) and fp8 (x16,
        # subtile dim = column half so DoubleRow identity pairs reach both)
        xgT16 = [top.tile([100, 4096], BF16, name=f"xg{d}", tag=f"xg{d}")
                 for d in range(2)]
        xgT8 = [top.tile([128, 2, 2048], FP8, name=f"xq{d}", tag=f"xq{d}")
                for d in range(2)]
        for d in range(2):
            nc.vector.memset(xgT8[d][96:128, :, :], 0.0)
        # H state, [100, 4 j, 258] with guard cols 0 and 257
        H16 = [[top.tile([100, 4, 258], BF16, name=f"H{l}{d}", tag=f"H{l}{d}")
                for d in range(2)] for l in range(2)]
        H8 = [[top.tile([128, 4, 258], FP8, name=f"G{l}{d}", tag=f"G{l}{d}")
               for d in range(2)] for l in range(2)]
        # only the guard columns (t=0 / t=257) need pre-zeroing; tile bodies
        # are written by the first sweep's rebuild before any read. On DVE to
        # keep the gpsimd DMA queue free for the embedding gathers.
        for l in range(2):
            for d in range(2):
                nc.vector.memset(H16[l][d][0:100, :, 0:1], 0.0)
                nc.vector.memset(H16[l][d][0:100, :, 257:258], 0.0)
                nc.vector.memset(H8[l][d][0:100, :, 0:1], 0.0)
                nc.vector.memset(H8[l][d][0:100, :, 257:258], 0.0)
                nc.vector.memset(H8[l][d][96:128, :, :], 0.0)
                nc.vector.memset(H8[l][d][96:101, :, :], 16.0)
        # edge weights
        uh_sb = top.tile([100, 800], BF16, name="uh", tag="uh")
        um_sb = top.tile([100, 800], BF16, name="um", tag="um")
        w2_sb = top.tile([100, 1], BF16, name="w2", tag="w2")
        b1_sb = top.tile([100, 1], F32, name="b1", tag="b1")
        b2_sb = top.tile([128, 1], F32, name="b2", tag="b2")
        selT_sb = top.tile([128, 64], F32, name="selT", tag="selT")
        xT = top.tile([100, 1024], BF16, name="xT", tag="xT")
        x8 = top.tile([128, 4, 256], FP8, name="x8", tag="x8")
        # fp16 input projections: shared tiles, wih0 for layer 0 (first 6400
        # cols), reloaded with wih1 for layer 1
        xp_sb = [top.tile([100, 12800], BF16, name=f"xp{d}", tag=f"xp{d}")
                 for d in range(2)]

        # =========== startup DMAs + embedding gather ===========
        w08ctx = tc.tile_pool(name="w08p", bufs=1)
        w08p = w08ctx.__enter__()
        wih08_sb = [w08p.tile([128, 16, 4, 128], FP8, name=f"w08_{d}",
                              tag=f"w08_{d}") for d in range(2)]
        with tc.tile_pool(name="embed", bufs=1) as epool, \
             tc.tile_pool(name="embps", bufs=2, space="PSUM") as eps:
            idx_sb = epool.tile([128, 4], I32, name="idx", tag="idx")
            nc.sync.dma_start(out=idx_sb[0:128, 0:4], in_=idx4d[0:128, 0:4])
            # fp8 weights for the critical-path GEMMs ride the ACT queue
            for d in range(2):
                nc.scalar.dma_start(out=wih08_sb[d][:, :, :, :], in_=wih08d[d])
            x_sb = epool.tile([128, 800], F32, name="xsb", tag="xsb")
            # word gathers first: transposes for ch 0-2 need only word data
            for tb in range(2):
                nc.gpsimd.indirect_dma_start(
                    out=x_sb[0:128, 400 * tb: 400 * tb + 300],
                    out_offset=None,
                    in_=wemb[:, :],
                    in_offset=IndirectOffsetOnAxis(
                        ap=idx_sb[0:128, tb:tb + 1], axis=0))
            for tb in range(2):
                nc.gpsimd.indirect_dma_start(
                    out=x_sb[0:128, 400 * tb + 300: 400 * tb + 400],
                    out_offset=None,
                    in_=pemb[:, :],
                    in_offset=IndirectOffsetOnAxis(
                        ap=idx_sb[0:128, 2 + tb:3 + tb], axis=0))
            nc.sync.dma_start(out=bias_sb[0][:, :], in_=bias0d[0])
            nc.sync.dma_start(out=idn8_sb[:, :, :], in_=idn8d[:, :, :])
            for d in range(2):
                nc.gpsimd.dma_start(out=whh8_sb[d][:, :, :, :], in_=whh8d[d])
            nc.sync.dma_start(out=bias_sb[1][:, :], in_=bias1d[0])
            nc.sync.dma_start(out=uh_sb[:, :], in_=uhd[:, :])
            nc.sync.dma_start(out=um_sb[:, :], in_=umd[:, :])
            nc.sync.dma_start(out=w2_sb[:, :], in_=w2d[:, :])
            nc.sync.dma_start(out=b1_sb[:, :], in_=b1d[:, :])
            nc.sync.dma_start(out=b2_sb[:, :], in_=b2d[:, :])
            nc.sync.dma_start(out=selT_sb[0:128, 0:64], in_=selTd[0:128, 0:64])
            # fp16 weights for layer 0: wih0 (filler GEMMs) + whh16
            for d in range(2):
                nc.scalar.dma_start(out=xp_sb[d][0:100, 0:6400], in_=wih0d[d])
            for d in range(2):
                nc.sync.dma_start(out=whh_sb[d][:, :], in_=whhd[d])
            # embed -> xT (transposes) -> x8; word chunks (ch 0-2) first
            for ch in range(4):
                for tb in range(2):
                    ptr = eps.tile([128, 128], F32, name="ptr", tag="ptr")
                    nc.tensor.transpose(
                        out=ptr[0:100, 0:128],
                        in_=x_sb[0:128, 400 * tb + 100 * ch: 400 * tb + 100 * ch + 100],
                        identity=idn128[:, :])
                    nc.vector.tensor_copy(
                        out=xT[0:100, 256 * ch + 128 * tb: 256 * ch + 128 * tb + 128],
                        in_=ptr[0:100, 0:128])
            nc.vector.memset(x8[96:128, :, :], 0.0)
            nc.vector.memset(x8[96:101, :, :], 16.0)
            nc.scalar.mul(x8[0:100, 0:2, 0:256], xT[0:100, 0:512], 16.0)
            nc.scalar.mul(x8[0:100, 2:4, 0:256], xT[0:100, 512:1024], 16.0)

        # =========== fp8 xg GEMM (layer l) -> xgT8 ===========
        def emit_xg8(l, wsb):
            npair = 2 if l == 0 else 4
            with tc.tile_pool(name=f"x8ps{l}", bufs=2, space="PSUM") as xps:
                for d in range(2):
                    for half in range(2):
                        ps = xps.tile([128, 2048], F32, name="x8ps", tag="x8ps")
                        for tl in range(8):
                            tt = 8 * half + tl
                            for p in range(npair):
                                if l == 0:
                                    rhs = x8[0:128, 2 * p: 2 * p + 2, 0:256]
                                else:
                                    dd, jp = divmod(p, 2)
                                    rhs = H8[0][dd][0:128, 2 * jp: 2 * jp + 2, 1:257]
                                nc.tensor.matmul(
                                    ps[0:100, 256 * tl: 256 * tl + 256],
                                    lhsT=wsb[d][0:128, tt, 2 * p: 2 * p + 2, 0:100],
                                    rhs=rhs,
                                    start=(p == 0), stop=(p == npair - 1),
                                    perf_mode=DR, skip_group_check=True)
                        # psum (x256) -> xgT8 (x16): only d0-half0 on ACT so
                        # sweep-0 d0's activations aren't queued behind d1
                        if half == 0 and d == 0:
                            nc.scalar.mul(xgT8[d][0:100, 0, 0:2048],
                                          ps[0:100, 0:2048], 1.0 / 16.0)
                        else:
                            nc.vector.tensor_scalar(
                                out=xgT8[d][0:100, half, 0:2048],
                                in0=ps[0:100, 0:2048],
                                scalar1=1.0 / 16.0, scalar2=None, op0=OP.mult)

        emit_xg8(0, wih08_sb)
        w08ctx.__exit__(None, None, None)

        # wih18 pool opens after wih08 closes (reuses the SBUF region)
        w18ctx = tc.tile_pool(name="w18p", bufs=1)
        w18p = w18ctx.__enter__()
        wih18_sb = [w18p.tile([128, 16, 8, 128], FP8, name=f"w18_{d}",
                              tag=f"w18_{d}") for d in range(2)]
        for d in range(2):
            nc.gpsimd.dma_start(out=wih18_sb[d][:, :, :, :], in_=wih18d[d])

        # =========== exact fp16 xg GEMM filler chunks ===========
        def xg16_chunks(l):
            """One gate-group quarter (d, q) of the exact xg per chunk, via a
            dedicated 2-bank psum tile. Emitted between sweep bursts as PE
            filler."""
            nch = 4 if l == 0 else 8
            chunks = []

            def mk(d, q):
                def emit(fps):
                    ps = fps.tile([128, 1024], F32, name="fxg", tag="fxg")
                    for tl in range(4):
                        tt = 4 * q + tl
                        for ch in range(nch):
                            if l == 0:
                                rhs = xT[0:100, 256 * ch: 256 * ch + 256]
                            else:
                                dd, j = divmod(ch, 4)
                                rhs = H16[0][dd][0:100, j, 1:257]
                            nc.tensor.matmul(
                                ps[0:100, 256 * tl: 256 * tl + 256],
                                lhsT=xp_sb[d][0:100, 100 * nch * tt + 100 * ch: 100 * nch * tt + 100 * ch + 100],
                                rhs=rhs,
                                start=(ch == 0), stop=False,
                                skip_group_check=True)
                        nc.tensor.matmul(
                            ps[0:100, 256 * tl: 256 * tl + 256],
                            lhsT=bias_sb[l][0:1, 1600 * d + 100 * (4 * q + tl): 1600 * d + 100 * (4 * q + tl) + 100],
                            rhs=ones_sb[0:1, 0:256],
                            start=False, stop=True, skip_group_check=True)
                    if q % 2 == 0:
                        nc.vector.tensor_copy(
                            out=xgT16[d][0:100, 1024 * q: 1024 * q + 1024],
                            in_=ps[0:100, 0:1024])
                    else:
                        nc.scalar.copy(
                            out=xgT16[d][0:100, 1024 * q: 1024 * q + 1024],
                            in_=ps[0:100, 0:1024])
                return emit
            for q in range(4):
                for d in range(2):
                    chunks.append(mk(d, q))
            return chunks

        # =========== Jacobi sweep emitter ===========
        # Stage-major emission interleaves both directions through the
        # in-order engine queues; gate psum is 4 quarter-tiles per direction
        # (one gate group each).
        def emit_sweeps(l):
            filler = xg16_chunks(l)
            with tc.tile_pool(name=f"sg{l}", bufs=1) as sgp, \
                 tc.tile_pool(name=f"scr{l}", bufs=1) as scr, \
                 tc.tile_pool(name=f"gps{l}", bufs=3, space="PSUM") as gps, \
                 tc.tile_pool(name=f"fps{l}", bufs=1, space="PSUM") as fps:
                for k, mode in enumerate(MODES):
                    nxt = MODES[k + 1] if k + 1 < K else '6'
                    sg_t, ps_t = {}, {}
                    # ---- PE stage ----
                    for d in range(2):
                        sg_t[d] = sgp.tile([100, 4096], BF16,
                                           name=f"sg{d}", tag=f"sg{d}")
                        if mode == 'x':
                            ps_t[d] = None
                            continue
                        o0 = 0 if d == 0 else 2
                        quarters = []
                        for q in range(4):
                            ps = gps.tile([128, 1024], F32, name="gps", tag="gps")
                            quarters.append(ps)
                            if mode == '6':
                                # xg preload: 3 on DVE, 1 on ACT (g-quarter)
                                if q != 1:
                                    nc.vector.tensor_copy(
                                        out=ps[0:100, 0:1024],
                                        in_=xgT16[d][0:100, 1024 * q: 1024 * q + 1024])
                                else:
                                    nc.scalar.copy(
                                        out=ps[0:100, 0:1024],
                                        in_=xgT16[d][0:100, 1024 * q: 1024 * q + 1024])
                            else:
                                # fp8 identity inject: (16I,0) for q<2 else (0,16I)
                                i0 = 0 if q < 2 else 2
                                for m in range(2):
                                    nc.tensor.matmul(
                                        ps[0:100, 512 * m: 512 * m + 512],
                                        lhsT=idn8_sb[0:128, i0:i0 + 2, 0:100],
                                        rhs=xgT8[d][0:128, 0:2, 1024 * (q % 2) + 512 * m: 1024 * (q % 2) + 512 * m + 512],
                                        start=True, stop=False,
                                        perf_mode=DR, skip_group_check=True)
                            for tl in range(4):
                                tt = 4 * q + tl
                                if mode in ('s', 't'):
                                    for p in range(2):
                                        nc.tensor.matmul(
                                            ps[0:100, 256 * tl: 256 * tl + 256],
                                            lhsT=whh8_sb[d][0:128, tt, 2 * p: 2 * p + 2, 0:100],
                                            rhs=H8[l][d][0:128, 2 * p: 2 * p + 2, o0: o0 + 256],
                                            start=False, stop=(p == 1),
                                            perf_mode=DR,
                                            skip_group_check=True)
                                else:
                                    for j in range(4):
                                        nc.tensor.matmul(
                                            ps[0:100, 256 * tl: 256 * tl + 256],
                                            lhsT=whh_sb[d][0:100, 400 * tt + 100 * j: 400 * tt + 100 * j + 100],
                                            rhs=H16[l][d][0:100, j, o0: o0 + 256],
                                            start=False, stop=(j == 3),
                                            skip_group_check=True)
                        ps_t[d] = quarters
                        # one exact-xg filler chunk per (sweep, dir) burst
                        if mode in ('x', 's', 't') and filler:
                            filler.pop(0)(fps)
                    # ---- ACT gate activations (i, g, f, o quarters) ----
                    funcs = (AF.Sigmoid, AF.Tanh, AF.Sigmoid, AF.Sigmoid)
                    for d in range(2):
                        sg = sg_t[d]
                        for q in range(4):
                            if mode == 'x':
                                src = xgT8[d][0:100, q // 2, 1024 * (q % 2): 1024 * (q % 2) + 1024]
                                sc = 1.0 / 16.0
                            else:
                                src = ps_t[d][q][0:100, 0:1024]
                                sc = INV_SCALE
                            nc.scalar.activation(
                                sg[0:100, 1024 * q: 1024 * q + 1024], src,
                                funcs[q], scale=sc)
                    # ---- DVE chain per dir (u -> scan -> [thc] -> H) ----
                    for d in range(2):
                        sg = sg_t[d]
                        u = scr.tile([100, 1024], BF16, name=f"u{d}", tag=f"u{d}")
                        c = scr.tile([100, 1024], F32, name=f"c{d}", tag=f"c{d}")
                        nc.vector.tensor_tensor(
                            out=u[0:100, 0:1024], in0=sg[0:100, 0:1024],
                            in1=sg[0:100, 1024:2048], op=OP.mult)
                        for j in range(4):
                            if d == 0:
                                nc.vector.tensor_tensor_scan(
                                    out=c[0:100, 256 * j: 256 * j + 256],
                                    data0=sg[0:100, 2048 + 256 * j: 2304 + 256 * j],
                                    data1=u[0:100, 256 * j: 256 * j + 256],
                                    initial=0.0, op0=OP.mult, op1=OP.add)
                            else:
                                e1 = 256 * j - 1
                                nc.vector.tensor_tensor_scan(
                                    out=c[0:100, 256 * j + 255: (e1 if e1 >= 0 else None): -1],
                                    data0=sg[0:100, 2303 + 256 * j: 2047 + 256 * j: -1],
                                    data1=u[0:100, 256 * j + 255: (e1 if e1 >= 0 else None): -1],
                                    initial=0.0, op0=OP.mult, op1=OP.add)
                        if mode in ('x', 's'):
                            th_ap = c[0:100, 0:1024]        # tanh(c) ~= c
                        else:
                            # u is dead after the scans: reuse it for tanh(c)
                            nc.scalar.activation(u[0:100, 0:1024],
                                                 c[0:100, 0:1024], AF.Tanh)
                            th_ap = u[0:100, 0:1024]
                        if nxt in ('s', 't'):
                            # H8 = (16*o) * th  [fp8, x16]
                            nc.vector.scalar_tensor_tensor(
                                out=H8[l][d][0:100, 0:4, 1:257],
                                in0=sg[0:100, 3072:4096], scalar=16.0,
                                in1=th_ap,
                                op0=OP.mult, op1=OP.mult)
                        else:
                            nc.vector.tensor_tensor(
                                out=H16[l][d][0:100, 0:4, 1:257],
                                in0=sg[0:100, 3072:4096], in1=th_ap,
                                op=OP.mult)
                # any filler chunks not yet emitted
                for chf in filler:
                    chf(fps)

        emit_sweeps(0)

        # =========== layer boundary ===========
        # layer-1 weight reloads first: their descriptor generation runs on
        # the ACT/SP/gpsimd queues while layer-0 finals are still in flight
        for d in range(2):
            nc.gpsimd.dma_start(out=whh8_sb[d][:, :, :, :], in_=whh8d[2 + d])
            nc.sync.dma_start(out=whh_sb[d][:, :], in_=whhd[2 + d])
        for d in range(2):
            nc.scalar.dma_start(out=xp_sb[d][0:100, 0:12800], in_=wih1d[d])
        # quantize final h0 into the (now free) H8[0] tiles for the fp8 xg1
        for dd in range(2):
            nc.scalar.mul(H8[0][dd][0:100, 0:4, 1:257],
                          H16[0][dd][0:100, 0:4, 1:257], 16.0)

        emit_xg8(1, wih18_sb)
        w18ctx.__exit__(None, None, None)

        emit_sweeps(1)

        # =========== edge scorer ===========
        with tc.tile_pool(name="edge", bufs=1) as ep, \
             tc.tile_pool(name="edgeth", bufs=4) as thp, \
             tc.tile_pool(name="edgeps", bufs=1, space="PSUM") as epps, \
             tc.tile_pool(name="edgept", bufs=1, space="PSUM") as ptps:
            # A^T [100 f, 256 t] first: it heads the serial select chain
            pA = epps.tile([128, 256], F32, name="pA", tag="pA")
            for c in range(8):
                dd, j = divmod(c, 4)
                nc.tensor.matmul(
                    pA[0:100, 0:256],
                    lhsT=uh_sb[0:100, 100 * c: 100 * c + 100],
                    rhs=H16[1][dd][0:100, j, 1:257],
                    start=(c == 0), stop=(c == 7))
            # B^T [100 f, 256 m] = Um^T @ h1cat (b1 folded into A side)
            pB = epps.tile([128, 256], F32, name="pB", tag="pB")
            for c in range(8):
                dd, j = divmod(c, 4)
                nc.tensor.matmul(
                    pB[0:100, 0:256],
                    lhsT=um_sb[0:100, 100 * c: 100 * c + 100],
                    rhs=H16[1][dd][0:100, j, 1:257],
                    start=(c == 0), stop=(c == 7))
            A_sb = ep.tile([100, 256], F32, name="A", tag="A")
            nc.vector.tensor_copy(out=A_sb[0:100, 0:256], in_=pA[0:100, 0:256])
            # select this core's 32 rows: transpose A^T chunks then selT matmul
            At_sb = ep.tile([128, 256], F32, name="At", tag="At")
            for m in range(2):
                pt = ptps.tile([128, 128], F32, name="pt", tag="pt")
                nc.tensor.transpose(
                    out=pt[0:128, 0:100],
                    in_=A_sb[0:100, 128 * m: 128 * m + 128],
                    identity=idn128[0:100, 0:100])
                nc.vector.tensor_copy(
                    out=At_sb[0:128, 128 * m: 128 * m + 100],
                    in_=pt[0:128, 0:100])
            # ATb [100 f, 32 r] and pBs = pB + b1 (fp16, SBUF)
            pS = ptps.tile([128, 32], F32, name="pS", tag="pS")
            for m in range(2):
                nc.tensor.matmul(
                    pS[0:100, 0:32],
                    lhsT=At_sb[0:128, 128 * m: 128 * m + 100],
                    rhs=selT_sb[0:128, 32 * m: 32 * m + 32],
                    start=(m == 0), stop=(m == 1))
            ATb = ep.tile([100, 32], F32, name="ATb", tag="ATb")
            nc.vector.tensor_copy(out=ATb[0:100, 0:32], in_=pS[0:100, 0:32])
            pBs = ep.tile([100, 256], BF16, name="pBs", tag="pBs")
            nc.vector.tensor_scalar(
                out=pBs[0:100, 0:256], in0=pB[0:100, 0:256],
                scalar1=b1_sb[0:100, 0:1], scalar2=None, op0=OP.add)

            psS_tiles = [epps.tile([128, 512], F32, name=f"psS{q}", tag=f"psS{q}")
                         for q in range(4)]
            for q in range(4):
                nc.vector.memset(psS_tiles[q][:, :], 0.0)
            gsb_tiles = [ep.tile([128, 512], F32, name=f"gsb{q}", tag=f"gsb{q}")
                         for q in range(4)]
            # batches of 4 rows: DVE builds tanh inputs in SBUF (2x mode),
            # one [100,1024] tanh per batch, then 4 score matmuls
            for rb4 in range(8):
                tin = thp.tile([100, 4, 256], BF16, name="tin", tag="tin")
                for rr in range(4):
                    r = 4 * rb4 + rr
                    nc.vector.tensor_scalar(
                        out=tin[0:100, rr, 0:256], in0=pBs[0:100, 0:256],
                        scalar1=ATb[0:100, r:r + 1], scalar2=None, op0=OP.add)
                th_t = thp.tile([100, 4, 256], BF16, name="th", tag="th")
                nc.scalar.activation(
                    th_t[0:100, 0:4, 0:256], tin[0:100, 0:4, 0:256], AF.Tanh)
                for rr in range(4):
                    r = 4 * rb4 + rr
                    q, half = divmod(r // 4, 2)
                    nc.tensor.matmul(
                        psS_tiles[q][32 * (r % 4): 32 * (r % 4) + 1,
                                     256 * half: 256 * half + 256],
                        lhsT=w2_sb[0:100, 0:1],
                        rhs=th_t[0:100, rr, 0:256],
                        start=True, stop=True,
                        skip_group_check=True,
                        tile_position=(0, 32 * (r % 4)))
                if rb4 % 2 == 1:
                    q = rb4 // 2
                    # quadrant q complete -> write back while later rows run
                    nc.vector.tensor_scalar(
                        out=gsb_tiles[q][0:128, 0:512],
                        in0=psS_tiles[q][0:128, 0:512],
                        scalar1=b2_sb[0:128, 0:1], scalar2=None, op0=OP.add)
                    for hh in range(2):
                        rb = 4 * (2 * q + hh)
                        nc.sync.dma_start(
                            out=grid[rb:rb + 4, 0:256],
                            in_=gsb_tiles[q][0:128:32, 256 * hh: 256 * hh + 256])

    nc.compile()
    return nc


_NC_CACHE = None


def _get_nc():
    global _NC_CACHE
    if _NC_CACHE is None:
        _NC_CACHE = build_nc()
    return _NC_CACHE


def kernel(**inputs) -> np.ndarray:
    from concourse.bass_utils import run_bass_kernel_spmd

    arr = _prep_inputs(**inputs)
    nc = _get_nc()
    in_maps = []
    for k in range(NC):
        m = dict(arr)
        m["selT"] = _make_selT(k)
        in_maps.append(m)
    res = run_bass_kernel_spmd(nc, in_maps, core_ids=list(range(NC)))
    grid = np.concatenate([res.results[k]["grid"] for k in range(NC)], axis=0)
    mask = np.ones((N, N), dtype=bool)
    np.fill_diagonal(mask, False)
    mask[:, 0] = False
    return grid[mask].reshape(-1, 1).astype(np.float32)


# revision 44
# speedup vs baseline: 1.0112x; 1.0112x over previous
"""Trainium2 Bass kernel: BiLSTM dependency-parser edge scorer.

Self-contained. Accepts FULL inputs (as produced by setup_inputs()), returns
the FULL [65025, 1] float32 score tensor.

Strategy (per NeuronCore, SPMD over 8 cores; replicated except the edge-score
row selection):
  - The LSTM recurrences are solved by Jacobi fixed-point iteration over the
    time-unrolled network: sweep k computes gates = xg + Whh @ H^(k-1) for ALL
    256 timesteps as batched matmuls (h-feedback lagged one sweep), applies
    sigmoid/tanh as wide activation ops, runs the c-recurrence with the DVE
    tensor_tensor_scan instruction, and rebuilds h = sigmoid(o) * tanh(c).
  - Intermediate sweeps run entirely in a x256-scaled fp8-e4m3 domain:
    recurrent matmuls in DoubleRow mode (two 100-row k-subtiles per
    instruction at 0.5 cyc/row), input projections xg pre-computed by a
    DoubleRow GEMM and stored as fp8 (x16); a (16*I, 0) DoubleRow identity
    copy injects them into the gate psum. tanh(c) is skipped (|c| < 0.45, so
    th := c). These approximations shift the intermediate fixed point by a
    contraction-healable amount only.
  - The final sweeps per layer run with exact fp16 xg (x256) and fp16
    recurrent matmuls, converging onto the true fixed point. The exact-xg
    GEMMs are emitted as PE filler chunks BETWEEN the intermediate sweeps'
    matmul bursts, so they stay off the critical path (and keep the PE
    p-state hot). Activations un-scale via the ACT scale operand (exact).
  - Gate layout: 16 tiles of 100 rows, tile = 4*gate_group + j, gate-group
    order (i, g, f, o); the gate psum is 4 quarter tiles per direction (one
    gate group each, 2 PSUM banks each), leaving 2 banks for filler GEMMs.
  - H is stored transposed ([100 hidden, 4 j-blocks, 258] with zero guard
    columns) in fp8 (x16, feeding DoubleRow sweeps) and fp16 (unscaled,
    feeding the fp16 sweeps, the layer-1 projections, and the edge GEMMs).
  - Edge MLP: scores[h,m] = w2 . tanh(A[h] + B[m] + b1) + b2 with
    A = h1 @ Uh^T, B = h1 @ Um^T. Each core computes a [32, 256] slice of the
    score grid (rows picked by a per-core one-hot input); host assembles.
"""

import os
import sys

sys.path.insert(0, "/opt/trn_rl_repo")

import numpy as np

import concourse.bass as bass
import concourse.mybir as mybir
from concourse import bacc
from concourse.bass import IndirectOffsetOnAxis
from concourse.masks import make_identity
from concourse.tile import TileContext

N = 256          # sequence length
NC = 8           # cores
F32 = mybir.dt.float32
BF16 = mybir.dt.float16
FP8 = mybir.dt.float8e4
I32 = mybir.dt.int32
AF = mybir.ActivationFunctionType
OP = mybir.AluOpType
DR = mybir.MatmulPerfMode.DoubleRow

# per-layer sweep schedule (DP_MODES, same string for both layers):
#   'x' = no recurrent matmul (gates = fp8 xg), tanh(c) skipped (th := c)
#   's' = fp8 DoubleRow recurrent matmul + fp8 xg, tanh(c) skipped
#   't' = fp8 DoubleRow recurrent matmul + fp8 xg, real tanh(c)
#   '6' = fp16 recurrent matmul + exact fp16 xg, real tanh(c)
MODES = os.environ.get("DP_MODES", "xssss6666")

SCALE = 256.0          # gate-psum scale
INV_SCALE = 1.0 / SCALE

# tile-group order (i, g, f, o)
_OG = (0, 2, 1, 3)


# ---------------------------------------------------------------------------
# host-side weight layout prep
# ---------------------------------------------------------------------------


def _bf(a):
    return np.ascontiguousarray(np.asarray(a).astype(np.float16))


def _f8(a):
    import ml_dtypes
    return np.ascontiguousarray(np.asarray(a).astype(ml_dtypes.float8_e4m3))


def _rows(tt):
    """Original gate-row indices (torch order i,f,g,o) for tile tt."""
    return 400 * _OG[tt // 4] + 100 * (tt % 4) + np.arange(100)


def _whh_lay(W):
    """W [1600, 400] -> [100 k, 6400] with free = 400*tt + 100*j + m."""
    out = np.zeros((100, 6400), np.float64)
    for tt in range(16):
        R = np.asarray(W, np.float64)[_rows(tt)]      # [100 m, 400]
        for j in range(4):
            out[:, 400 * tt + 100 * j: 400 * tt + 100 * j + 100] = \
                R[:, 100 * j: 100 * j + 100].T
    return out


def _wih_lay(W, nch):
    """W [1600, 100*nch] -> [100 k, .]: free = (100*nch)*tt + 100*ch + m."""
    D = 100 * nch
    out = np.zeros((100, 16 * D), np.float64)
    for tt in range(16):
        R = np.asarray(W, np.float64)[_rows(tt)]      # [100 m, D]
        for ch in range(nch):
            out[:, D * tt + 100 * ch: D * tt + 100 * ch + 100] = \
                R[:, 100 * ch: 100 * ch + 100].T
    return out


def _bias_lay(b):
    """b [1600] -> [1600] with index 100*tt + m."""
    out = np.zeros(1600, np.float64)
    for tt in range(16):
        out[100 * tt: 100 * tt + 100] = np.asarray(b, np.float64)[_rows(tt)]
    return out


def _prep_inputs(word_idx, pos_idx, word_emb, pos_emb,
                 Wih0, Whh0, bih0, bhh0, Wih1, Whh1, bih1, bhh1,
                 fc1_W, fc1_b, fc2_W, fc2_b):
    arr = {}
    wi = np.asarray(word_idx).reshape(N).astype(np.int32)
    pi = np.asarray(pos_idx).reshape(N).astype(np.int32)
    arr["idx4"] = np.ascontiguousarray(
        np.stack([wi[:128], wi[128:], pi[:128], pi[128:]], axis=1))
    arr["wemb"] = np.ascontiguousarray(np.asarray(word_emb, dtype=np.float32))
    arr["pemb"] = np.ascontiguousarray(np.asarray(pos_emb, dtype=np.float32))

    whh = np.zeros((4, 100, 6400), np.float64)
    wih0 = np.zeros((2, 100, 6400), np.float64)
    wih1 = np.zeros((2, 100, 12800), np.float64)
    bias = np.zeros((2, 3200), np.float64)
    for d in range(2):
        whh[2 * 0 + d] = _whh_lay(np.asarray(Whh0)[d])
        whh[2 * 1 + d] = _whh_lay(np.asarray(Whh1)[d])
        wih0[d] = _wih_lay(np.asarray(Wih0)[d], 4)
        wih1[d] = _wih_lay(np.asarray(Wih1)[d], 8)
        bias[0, 1600 * d: 1600 * d + 1600] = _bias_lay(
            np.asarray(bih0)[d] + np.asarray(bhh0)[d])
        bias[1, 1600 * d: 1600 * d + 1600] = _bias_lay(
            np.asarray(bih1)[d] + np.asarray(bhh1)[d])
    # fp16 weights carry the full 256x psum scale (their rhs are unscaled)
    arr["whh"] = _bf(whh * SCALE)
    arr["wih0"] = _bf(wih0 * SCALE)
    arr["wih1"] = _bf(wih1 * SCALE)
    arr["bias0"] = _bf(bias[0:1] * SCALE)
    arr["bias1"] = _bf(bias[1:2] * SCALE)
    # fp8 weights carry 16x (their rhs carry the other 16x)
    # DoubleRow weights need a 32-aligned subtile stride: pad M 100 -> 128
    whh8p = np.zeros((4, 128, 16, 4, 128), np.float64)
    whh8p[:, 0:100, :, :, 0:100] = (whh * 16.0).reshape(4, 100, 16, 4, 100)
    arr["whh8"] = _f8(whh8p)
    w08 = np.zeros((2, 128, 16, 4, 128), np.float64)
    w08[:, 0:100, :, :, 0:100] = (wih0 * 16.0).reshape(2, 100, 16, 4, 100)
    w18 = np.zeros((2, 128, 16, 8, 128), np.float64)
    w18[:, 0:100, :, :, 0:100] = (wih1 * 16.0).reshape(2, 100, 16, 8, 100)
    for d in range(2):
        for tt in range(16):
            w08[d, 100, tt, 0, 0:100] = bias[0, 1600 * d + 100 * tt:
                                             1600 * d + 100 * tt + 100] * 16.0
            w18[d, 100, tt, 0, 0:100] = bias[1, 1600 * d + 100 * tt:
                                             1600 * d + 100 * tt + 100] * 16.0
    arr["wih08"] = _f8(w08)
    arr["wih18"] = _f8(w18)
    # DoubleRow identity pairs: rows (16I, 0, 0, 16I) -> (16I,0) and (0,16I)
    idn8 = np.zeros((128, 4, 128), np.float64)
    idn8[0:100, 0, 0:100] = 16.0 * np.eye(100)
    idn8[0:100, 3, 0:100] = 16.0 * np.eye(100)
    arr["idn8"] = _f8(idn8)

    # edge MLP: Uh = fc1_W[:, :800].T chunks, Um = fc1_W[:, 800:].T chunks
    f1 = np.asarray(fc1_W, np.float64)
    uh = np.zeros((100, 800), np.float64)
    um = np.zeros((100, 800), np.float64)
    for c in range(8):
        uh[:, 100 * c: 100 * c + 100] = f1[:, 100 * c: 100 * c + 100].T
        um[:, 100 * c: 100 * c + 100] = f1[:, 800 + 100 * c: 900 + 100 * c].T
    arr["uh"] = _bf(uh)
    arr["um"] = _bf(um)
    arr["w2"] = _bf(np.asarray(fc2_W, np.float64).reshape(100, 1))
    arr["b1"] = np.ascontiguousarray(
        np.asarray(fc1_b, np.float32).reshape(100, 1))
    arr["b2"] = np.ascontiguousarray(
        np.full((128, 1), np.float32(np.asarray(fc2_b).reshape(())),
                dtype=np.float32))
    return arr


def _make_selT(core):
    s = np.zeros((2, 128, 32), np.float32)
    for r in range(32):
        t = 32 * core + r
        s[t // 128, t % 128, r] = 1.0
    return np.ascontiguousarray(np.concatenate([s[0], s[1]], axis=1))


# ---------------------------------------------------------------------------
# device kernel build
# ---------------------------------------------------------------------------


def build_nc():
    nc = bacc.Bacc("TRN2", target_bir_lowering=False, debug=False,
                   num_devices=NC)
    wemb = nc.dram_tensor("wemb", [50000, 300], F32, kind="ExternalInput").ap()
    pemb = nc.dram_tensor("pemb", [50, 100], F32, kind="ExternalInput").ap()
    idx4d = nc.dram_tensor("idx4", [128, 4], I32, kind="ExternalInput").ap()
    whhd = nc.dram_tensor("whh", [4, 100, 6400], BF16, kind="ExternalInput").ap()
    whh8d = nc.dram_tensor("whh8", [4, 128, 16, 4, 128], FP8, kind="ExternalInput").ap()
    wih0d = nc.dram_tensor("wih0", [2, 100, 6400], BF16, kind="ExternalInput").ap()
    wih1d = nc.dram_tensor("wih1", [2, 100, 12800], BF16, kind="ExternalInput").ap()
    wih08d = nc.dram_tensor("wih08", [2, 128, 16, 4, 128], FP8, kind="ExternalInput").ap()
    wih18d = nc.dram_tensor("wih18", [2, 128, 16, 8, 128], FP8, kind="ExternalInput").ap()
    bias0d = nc.dram_tensor("bias0", [1, 3200], BF16, kind="ExternalInput").ap()
    bias1d = nc.dram_tensor("bias1", [1, 3200], BF16, kind="ExternalInput").ap()
    idn8d = nc.dram_tensor("idn8", [128, 4, 128], FP8, kind="ExternalInput").ap()
    uhd = nc.dram_tensor("uh", [100, 800], BF16, kind="ExternalInput").ap()
    umd = nc.dram_tensor("um", [100, 800], BF16, kind="ExternalInput").ap()
    w2d = nc.dram_tensor("w2", [100, 1], BF16, kind="ExternalInput").ap()
    b1d = nc.dram_tensor("b1", [100, 1], F32, kind="ExternalInput").ap()
    b2d = nc.dram_tensor("b2", [128, 1], F32, kind="ExternalInput").ap()
    selTd = nc.dram_tensor("selT", [128, 64], F32, kind="ExternalInput").ap()
    grid = nc.dram_tensor("grid", [32, N], F32, kind="ExternalOutput").ap()

    K = len(MODES)

    from contextlib import ExitStack
    with TileContext(nc) as tc, ExitStack() as ctx:
        top = ctx.enter_context(tc.tile_pool(name="top", bufs=1))
        # recurrent weights: 2 shared tiles per format, reloaded for layer 1
        whh_sb = [top.tile([100, 6400], BF16, name=f"whh{d}", tag=f"whh{d}")
                  for d in range(2)]
        whh8_sb = [top.tile([128, 16, 4, 128], FP8, name=f"wh8{d}", tag=f"wh8{d}")
                   for d in range(2)]
        bias_sb = [top.tile([1, 3200], BF16, name=f"bias{l}", tag=f"bias{l}")
                   for l in range(2)]
        idn8_sb = top.tile([128, 4, 128], FP8, name="idn8", tag="idn8")
        idn128 = top.tile([128, 128], F32, name="idn128", tag="idn128")
        make_identity(nc, idn128[:, :])
        ones_sb = top.tile([1, N], BF16, name="ones", tag="ones")
        nc.vector.memset(ones_sb[:, :], 1.0)
        # xg tiles, shared between layers: fp16 exact (x256) and fp8 (x16,
        # subtile dim = column half so DoubleRow identity pairs reach both)
        xgT16 = [top.tile([100, 4096], BF16, name=f"xg{d}", tag=f"xg{d}")
                 for d in range(2)]
        xgT8 = [top.tile([128, 2, 2048], FP8, name=f"xq{d}", tag=f"xq{d}")
                for d in range(2)]
        for d in range(2):
            nc.vector.memset(xgT8[d][96:128, :, :], 0.0)
        # H state, [100, 4 j, 258] with guard cols 0 and 257
        H16 = [[top.tile([100, 4, 258], BF16, name=f"H{l}{d}", tag=f"H{l}{d}")
                for d in range(2)] for l in range(2)]
        H8 = [[top.tile([128, 4, 258], FP8, name=f"G{l}{d}", tag=f"G{l}{d}")
               for d in range(2)] for l in range(2)]
        # only the guard columns (t=0 / t=257) need pre-zeroing; tile bodies
        # are written by the first sweep's rebuild before any read. On DVE to
        # keep the gpsimd DMA queue free for the embedding gathers.
        for l in range(2):
            for d in range(2):
                nc.vector.memset(H16[l][d][0:100, :, 0:1], 0.0)
                nc.vector.memset(H16[l][d][0:100, :, 257:258], 0.0)
                nc.vector.memset(H8[l][d][0:100, :, 0:1], 0.0)
                nc.vector.memset(H8[l][d][0:100, :, 257:258], 0.0)
                nc.vector.memset(H8[l][d][96:128, :, :], 0.0)
                nc.vector.memset(H8[l][d][96:101, :, :], 16.0)
        # edge weights
        uh_sb = top.tile([100, 800], BF16, name="uh", tag="uh")
        um_sb = top.tile([100, 800], BF16, name="um", tag="um")
        w2_sb = top.tile([100, 1], BF16, name="w2", tag="w2")
        b1_sb = top.tile([100, 1], F32, name="b1", tag="b1")
        b2_sb = top.tile([128, 1], F32, name="b2", tag="b2")
        selT_sb = top.tile([128, 64], F32, name="selT", tag="selT")
        xT = top.tile([100, 1024], BF16, name="xT", tag="xT")
        x8 = top.tile([128, 4, 256], FP8, name="x8", tag="x8")
        # fp16 input projections: shared tiles, wih0 for layer 0 (first 6400
        # cols), reloaded with wih1 for layer 1
        xp_sb = [top.tile([100, 12800], BF16, name=f"xp{d}", tag=f"xp{d}")
                 for d in range(2)]

        # =========== startup DMAs + embedding gather ===========
        w08ctx = tc.tile_pool(name="w08p", bufs=1)
        w08p = w08ctx.__enter__()
        wih08_sb = [w08p.tile([128, 16, 4, 128], FP8, name=f"w08_{d}",
                              tag=f"w08_{d}") for d in range(2)]
        with tc.tile_pool(name="embed", bufs=1) as epool, \
             tc.tile_pool(name="embps", bufs=2, space="PSUM") as eps:
            idx_sb = epool.tile([128, 4], I32, name="idx", tag="idx")
            nc.sync.dma_start(out=idx_sb[0:128, 0:4], in_=idx4d[0:128, 0:4])
            # fp8 weights for the critical-path GEMMs ride the ACT queue
            for d in range(2):
                nc.scalar.dma_start(out=wih08_sb[d][:, :, :, :], in_=wih08d[d])
            x_sb = epool.tile([128, 800], F32, name="xsb", tag="xsb")
            # word gathers first: transposes for ch 0-2 need only word data
            for tb in range(2):
                nc.gpsimd.indirect_dma_start(
                    out=x_sb[0:128, 400 * tb: 400 * tb + 300],
                    out_offset=None,
                    in_=wemb[:, :],
                    in_offset=IndirectOffsetOnAxis(
                        ap=idx_sb[0:128, tb:tb + 1], axis=0))
            for tb in range(2):
                nc.gpsimd.indirect_dma_start(
                    out=x_sb[0:128, 400 * tb + 300: 400 * tb + 400],
                    out_offset=None,
                    in_=pemb[:, :],
                    in_offset=IndirectOffsetOnAxis(
                        ap=idx_sb[0:128, 2 + tb:3 + tb], axis=0))
            nc.sync.dma_start(out=bias_sb[0][:, :], in_=bias0d[0])
            nc.sync.dma_start(out=idn8_sb[:, :, :], in_=idn8d[:, :, :])
            for d in range(2):
                nc.gpsimd.dma_start(out=whh8_sb[d][:, :, :, :], in_=whh8d[d])
            nc.sync.dma_start(out=bias_sb[1][:, :], in_=bias1d[0])
            nc.sync.dma_start(out=uh_sb[:, :], in_=uhd[:, :])
            nc.sync.dma_start(out=um_sb[:, :], in_=umd[:, :])
            nc.sync.dma_start(out=w2_sb[:, :], in_=w2d[:, :])
            nc.sync.dma_start(out=b1_sb[:, :], in_=b1d[:, :])
            nc.sync.dma_start(out=b2_sb[:, :], in_=b2d[:, :])
            nc.sync.dma_start(out=selT_sb[0:128, 0:64], in_=selTd[0:128, 0:64])
            # fp16 weights for layer 0: wih0 (filler GEMMs) + whh16
            for d in range(2):
                nc.scalar.dma_start(out=xp_sb[d][0:100, 0:6400], in_=wih0d[d])
            for d in range(2):
                nc.sync.dma_start(out=whh_sb[d][:, :], in_=whhd[d])
            # embed -> xT (transposes) -> x8; word chunks (ch 0-2) first
            for ch in range(4):
                for tb in range(2):
                    ptr = eps.tile([128, 128], F32, name="ptr", tag="ptr")
                    nc.tensor.transpose(
                        out=ptr[0:100, 0:128],
                        in_=x_sb[0:128, 400 * tb + 100 * ch: 400 * tb + 100 * ch + 100],
                        identity=idn128[:, :])
                    nc.vector.tensor_copy(
                        out=xT[0:100, 256 * ch + 128 * tb: 256 * ch + 128 * tb + 128],
                        in_=ptr[0:100, 0:128])
            nc.vector.memset(x8[96:128, :, :], 0.0)
            nc.vector.memset(x8[96:101, :, :], 16.0)
            nc.scalar.mul(x8[0:100, 0:2, 0:256], xT[0:100, 0:512], 16.0)
            nc.scalar.mul(x8[0:100, 2:4, 0:256], xT[0:100, 512:1024], 16.0)

        # =========== fp8 xg GEMM (layer l) -> xgT8 ===========
        def emit_xg8(l, wsb):
            npair = 2 if l == 0 else 4
            with tc.tile_pool(name=f"x8ps{l}", bufs=2, space="PSUM") as xps:
                for d in range(2):
                    for half in range(2):
                        ps = xps.tile([128, 2048], F32, name="x8ps", tag="x8ps")
                        for tl in range(8):
                            tt = 8 * half + tl
                            for p in range(npair):
                                if l == 0:
                                    rhs = x8[0:128, 2 * p: 2 * p + 2, 0:256]
                                else:
                                    dd, jp = divmod(p, 2)
                                    rhs = H8[0][dd][0:128, 2 * jp: 2 * jp + 2, 1:257]
                                nc.tensor.matmul(
                                    ps[0:100, 256 * tl: 256 * tl + 256],
                                    lhsT=wsb[d][0:128, tt, 2 * p: 2 * p + 2, 0:100],
                                    rhs=rhs,
                                    start=(p == 0), stop=(p == npair - 1),
                                    perf_mode=DR, skip_group_check=True)
                        # psum (x256) -> xgT8 (x16)
                        if half == 0:
                            nc.scalar.mul(xgT8[d][0:100, 0, 0:2048],
                                          ps[0:100, 0:2048], 1.0 / 16.0)
                        else:
                            nc.vector.tensor_scalar(
                                out=xgT8[d][0:100, 1, 0:2048],
                                in0=ps[0:100, 0:2048],
                                scalar1=1.0 / 16.0, scalar2=None, op0=OP.mult)

        emit_xg8(0, wih08_sb)
        w08ctx.__exit__(None, None, None)

        # wih18 pool opens after wih08 closes (reuses the SBUF region)
        w18ctx = tc.tile_pool(name="w18p", bufs=1)
        w18p = w18ctx.__enter__()
        wih18_sb = [w18p.tile([128, 16, 8, 128], FP8, name=f"w18_{d}",
                              tag=f"w18_{d}") for d in range(2)]
        for d in range(2):
            nc.gpsimd.dma_start(out=wih18_sb[d][:, :, :, :], in_=wih18d[d])

        # =========== exact fp16 xg GEMM filler chunks ===========
        def xg16_chunks(l):
            """One gate-group quarter (d, q) of the exact xg per chunk, via a
            dedicated 2-bank psum tile. Emitted between sweep bursts as PE
            filler."""
            nch = 4 if l == 0 else 8
            chunks = []

            def mk(d, q):
                def emit(fps):
                    ps = fps.tile([128, 1024], F32, name="fxg", tag="fxg")
                    for tl in range(4):
                        tt = 4 * q + tl
                        for ch in range(nch):
                            if l == 0:
                                rhs = xT[0:100, 256 * ch: 256 * ch + 256]
                            else:
                                dd, j = divmod(ch, 4)
                                rhs = H16[0][dd][0:100, j, 1:257]
                            nc.tensor.matmul(
                                ps[0:100, 256 * tl: 256 * tl + 256],
                                lhsT=xp_sb[d][0:100, 100 * nch * tt + 100 * ch: 100 * nch * tt + 100 * ch + 100],
                                rhs=rhs,
                                start=(ch == 0), stop=False,
                                skip_group_check=True)
                        nc.tensor.matmul(
                            ps[0:100, 256 * tl: 256 * tl + 256],
                            lhsT=bias_sb[l][0:1, 1600 * d + 100 * (4 * q + tl): 1600 * d + 100 * (4 * q + tl) + 100],
                            rhs=ones_sb[0:1, 0:256],
                            start=False, stop=True, skip_group_check=True)
                    if q % 2 == 0:
                        nc.vector.tensor_copy(
                            out=xgT16[d][0:100, 1024 * q: 1024 * q + 1024],
                            in_=ps[0:100, 0:1024])
                    else:
                        nc.scalar.copy(
                            out=xgT16[d][0:100, 1024 * q: 1024 * q + 1024],
                            in_=ps[0:100, 0:1024])
                return emit
            for q in range(4):
                for d in range(2):
                    chunks.append(mk(d, q))
            return chunks

        # =========== Jacobi sweep emitter ===========
        # Stage-major emission interleaves both directions through the
        # in-order engine queues; gate psum is 4 quarter-tiles per direction
        # (one gate group each).
        def emit_sweeps(l):
            filler = xg16_chunks(l)
            with tc.tile_pool(name=f"sg{l}", bufs=1) as sgp, \
                 tc.tile_pool(name=f"scr{l}", bufs=1) as scr, \
                 tc.tile_pool(name=f"gps{l}", bufs=3, space="PSUM") as gps, \
                 tc.tile_pool(name=f"fps{l}", bufs=1, space="PSUM") as fps:
                for k, mode in enumerate(MODES):
                    nxt = MODES[k + 1] if k + 1 < K else '6'
                    sg_t, ps_t = {}, {}
                    # ---- PE stage ----
                    for d in range(2):
                        sg_t[d] = sgp.tile([100, 4096], BF16,
                                           name=f"sg{d}", tag=f"sg{d}")
                        if mode == 'x':
                            ps_t[d] = None
                            continue
                        o0 = 0 if d == 0 else 2
                        quarters = []
                        for q in range(4):
                            ps = gps.tile([128, 1024], F32, name="gps", tag="gps")
                            quarters.append(ps)
                            if mode == '6':
                                # xg preload: 3 on DVE, 1 on ACT (g-quarter)
                                if q != 1:
                                    nc.vector.tensor_copy(
                                        out=ps[0:100, 0:1024],
                                        in_=xgT16[d][0:100, 1024 * q: 1024 * q + 1024])
                                else:
                                    nc.scalar.copy(
                                        out=ps[0:100, 0:1024],
                                        in_=xgT16[d][0:100, 1024 * q: 1024 * q + 1024])
                            else:
                                # fp8 identity inject: (16I,0) for q<2 else (0,16I)
                                i0 = 0 if q < 2 else 2
                                for m in range(2):
                                    nc.tensor.matmul(
                                        ps[0:100, 512 * m: 512 * m + 512],
                                        lhsT=idn8_sb[0:128, i0:i0 + 2, 0:100],
                                        rhs=xgT8[d][0:128, 0:2, 1024 * (q % 2) + 512 * m: 1024 * (q % 2) + 512 * m + 512],
                                        start=True, stop=False,
                                        perf_mode=DR, skip_group_check=True)
                            for tl in range(4):
                                tt = 4 * q + tl
                                if mode in ('s', 't'):
                                    for p in range(2):
                                        nc.tensor.matmul(
                                            ps[0:100, 256 * tl: 256 * tl + 256],
                                            lhsT=whh8_sb[d][0:128, tt, 2 * p: 2 * p + 2, 0:100],
                                            rhs=H8[l][d][0:128, 2 * p: 2 * p + 2, o0: o0 + 256],
                                            start=False, stop=(p == 1),
                                            perf_mode=DR,
                                            skip_group_check=True)
                                else:
                                    for j in range(4):
                                        nc.tensor.matmul(
                                            ps[0:100, 256 * tl: 256 * tl + 256],
                                            lhsT=whh_sb[d][0:100, 400 * tt + 100 * j: 400 * tt + 100 * j + 100],
                                            rhs=H16[l][d][0:100, j, o0: o0 + 256],
                                            start=False, stop=(j == 3),
                                            skip_group_check=True)
                        ps_t[d] = quarters
                        # one exact-xg filler chunk per (sweep, dir) burst
                        if mode in ('x', 's', 't') and filler:
                            filler.pop(0)(fps)
                    # ---- ACT gate activations (i, g, f, o quarters) ----
                    funcs = (AF.Sigmoid, AF.Tanh, AF.Sigmoid, AF.Sigmoid)
                    for d in range(2):
                        sg = sg_t[d]
                        for q in range(4):
                            if mode == 'x':
                                src = xgT8[d][0:100, q // 2, 1024 * (q % 2): 1024 * (q % 2) + 1024]
                                sc = 1.0 / 16.0
                            else:
                                src = ps_t[d][q][0:100, 0:1024]
                                sc = INV_SCALE
                            nc.scalar.activation(
                                sg[0:100, 1024 * q: 1024 * q + 1024], src,
                                funcs[q], scale=sc)
                    # ---- DVE chain per dir (u -> scan -> [thc] -> H) ----
                    for d in range(2):
                        sg = sg_t[d]
                        u = scr.tile([100, 1024], BF16, name=f"u{d}", tag=f"u{d}")
                        c = scr.tile([100, 1024], F32, name=f"c{d}", tag=f"c{d}")
                        nc.vector.tensor_tensor(
                            out=u[0:100, 0:1024], in0=sg[0:100, 0:1024],
                            in1=sg[0:100, 1024:2048], op=OP.mult)
                        for j in range(4):
                            if d == 0:
                                nc.vector.tensor_tensor_scan(
                                    out=c[0:100, 256 * j: 256 * j + 256],
                                    data0=sg[0:100, 2048 + 256 * j: 2304 + 256 * j],
                                    data1=u[0:100, 256 * j: 256 * j + 256],
                                    initial=0.0, op0=OP.mult, op1=OP.add)
                            else:
                                e1 = 256 * j - 1
                                nc.vector.tensor_tensor_scan(
                                    out=c[0:100, 256 * j + 255: (e1 if e1 >= 0 else None): -1],
                                    data0=sg[0:100, 2303 + 256 * j: 2047 + 256 * j: -1],
                                    data1=u[0:100, 256 * j + 255: (e1 if e1 >= 0 else None): -1],
                                    initial=0.0, op0=OP.mult, op1=OP.add)
                        if mode in ('x', 's'):
                            th_ap = c[0:100, 0:1024]        # tanh(c) ~= c
                        else:
                            # u is dead after the scans: reuse it for tanh(c)
                            nc.scalar.activation(u[0:100, 0:1024],
                                                 c[0:100, 0:1024], AF.Tanh)
                            th_ap = u[0:100, 0:1024]
                        if nxt in ('s', 't'):
                            # H8 = (16*o) * th  [fp8, x16]
                            nc.vector.scalar_tensor_tensor(
                                out=H8[l][d][0:100, 0:4, 1:257],
                                in0=sg[0:100, 3072:4096], scalar=16.0,
                                in1=th_ap,
                                op0=OP.mult, op1=OP.mult)
                        else:
                            nc.vector.tensor_tensor(
                                out=H16[l][d][0:100, 0:4, 1:257],
                                in0=sg[0:100, 3072:4096], in1=th_ap,
                                op=OP.mult)
                # any filler chunks not yet emitted
                for chf in filler:
                    chf(fps)

        emit_sweeps(0)

        # =========== layer boundary ===========
        # layer-1 weight reloads first: their descriptor generation runs on
        # the ACT/SP/gpsimd queues while layer-0 finals are still in flight
        for d in range(2):
            nc.gpsimd.dma_start(out=whh8_sb[d][:, :, :, :], in_=whh8d[2 + d])
            nc.sync.dma_start(out=whh_sb[d][:, :], in_=whhd[2 + d])
        for d in range(2):
            nc.scalar.dma_start(out=xp_sb[d][0:100, 0:12800], in_=wih1d[d])
        # quantize final h0 into the (now free) H8[0] tiles for the fp8 xg1
        for dd in range(2):
            nc.scalar.mul(H8[0][dd][0:100, 0:4, 1:257],
                          H16[0][dd][0:100, 0:4, 1:257], 16.0)

        emit_xg8(1, wih18_sb)
        w18ctx.__exit__(None, None, None)

        emit_sweeps(1)

        # =========== edge scorer ===========
        with tc.tile_pool(name="edge", bufs=1) as ep, \
             tc.tile_pool(name="edgeth", bufs=4) as thp, \
             tc.tile_pool(name="edgeps", bufs=1, space="PSUM") as epps, \
             tc.tile_pool(name="edgept", bufs=1, space="PSUM") as ptps:
            # A^T [100 f, 256 t] first: it heads the serial select chain
            pA = epps.tile([128, 256], F32, name="pA", tag="pA")
            for c in range(8):
                dd, j = divmod(c, 4)
                nc.tensor.matmul(
                    pA[0:100, 0:256],
                    lhsT=uh_sb[0:100, 100 * c: 100 * c + 100],
                    rhs=H16[1][dd][0:100, j, 1:257],
                    start=(c == 0), stop=(c == 7))
            # B^T [100 f, 256 m] = Um^T @ h1cat (b1 folded into A side)
            pB = epps.tile([128, 256], F32, name="pB", tag="pB")
            for c in range(8):
                dd, j = divmod(c, 4)
                nc.tensor.matmul(
                    pB[0:100, 0:256],
                    lhsT=um_sb[0:100, 100 * c: 100 * c + 100],
                    rhs=H16[1][dd][0:100, j, 1:257],
                    start=(c == 0), stop=(c == 7))
            A_sb = ep.tile([100, 256], F32, name="A", tag="A")
            nc.vector.tensor_copy(out=A_sb[0:100, 0:256], in_=pA[0:100, 0:256])
            # select this core's 32 rows: transpose A^T chunks then selT matmul
            At_sb = ep.tile([128, 256], F32, name="At", tag="At")
            for m in range(2):
                pt = ptps.tile([128, 128], F32, name="pt", tag="pt")
                nc.tensor.transpose(
                    out=pt[0:128, 0:100],
                    in_=A_sb[0:100, 128 * m: 128 * m + 128],
                    identity=idn128[0:100, 0:100])
                nc.vector.tensor_copy(
                    out=At_sb[0:128, 128 * m: 128 * m + 100],
                    in_=pt[0:128, 0:100])
            # ATb [100 f, 32 r] and pBs = pB + b1 (fp16, SBUF)
            pS = ptps.tile([128, 32], F32, name="pS", tag="pS")
            for m in range(2):
                nc.tensor.matmul(
                    pS[0:100, 0:32],
                    lhsT=At_sb[0:128, 128 * m: 128 * m + 100],
                    rhs=selT_sb[0:128, 32 * m: 32 * m + 32],
                    start=(m == 0), stop=(m == 1))
            ATb = ep.tile([100, 32], F32, name="ATb", tag="ATb")
            nc.vector.tensor_copy(out=ATb[0:100, 0:32], in_=pS[0:100, 0:32])
            pBs = ep.tile([100, 256], BF16, name="pBs", tag="pBs")
            nc.vector.tensor_scalar(
                out=pBs[0:100, 0:256], in0=pB[0:100, 0:256],
                scalar1=b1_sb[0:100, 0:1], scalar2=None, op0=OP.add)

            psS_tiles = [epps.tile([128, 512], F32, name=f"psS{q}", tag=f"psS{q}")
                         for q in range(4)]
            for q in range(4):
                nc.vector.memset(psS_tiles[q][:, :], 0.0)
            gsb_tiles = [ep.tile([128, 512], F32, name=f"gsb{q}", tag=f"gsb{q}")
                         for q in range(4)]
            # batches of 4 rows: DVE builds tanh inputs in SBUF (2x mode),
            # one [100,1024] tanh per batch, then 4 score matmuls
            for rb4 in range(8):
                tin = thp.tile([100, 4, 256], BF16, name="tin", tag="tin")
                for rr in range(4):
                    r = 4 * rb4 + rr
                    nc.vector.tensor_scalar(
                        out=tin[0:100, rr, 0:256], in0=pBs[0:100, 0:256],
                        scalar1=ATb[0:100, r:r + 1], scalar2=None, op0=OP.add)
                th_t = thp.tile([100, 4, 256], BF16, name="th", tag="th")
                nc.scalar.activation(
                    th_t[0:100, 0:4, 0:256], tin[0:100, 0:4, 0:256], AF.Tanh)
                for rr in range(4):
                    r = 4 * rb4 + rr
                    q, half = divmod(r // 4, 2)
                    nc.tensor.matmul(
                        psS_tiles[q][32 * (r % 4): 32 * (r % 4) + 1,
                                     256 * half: 256 * half + 256],
                        lhsT=w2_sb[0:100, 0:1],
                        rhs=th_t[0:100, rr, 0:256],
                        start=True, stop=True,
                        skip_group_check=True,
                        tile_position=(0, 32 * (r % 4)))
                if rb4 % 2 == 1:
                    q = rb4 // 2
                    # quadrant q complete -> write back while later rows run
                    nc.vector.tensor_scalar(
                        out=gsb_tiles[q][0:128, 0:512],
                        in0=psS_tiles[q][0:128, 0:512],
                        scalar1=b2_sb[0:128, 0:1], scalar2=None, op0=OP.add)
                    for hh in range(2):
                        rb = 4 * (2 * q + hh)
                        nc.sync.dma_start(
                            out=grid[rb:rb + 4, 0:256],
                            in_=gsb_tiles[q][0:128:32, 256 * hh: 256 * hh + 256])

    nc.compile()
    return nc


_NC_CACHE = None


def _get_nc():
    global _NC_CACHE
    if _NC_CACHE is None:
        _NC_CACHE = build_nc()
    return _NC_CACHE


def kernel(**inputs) -> np.ndarray:
    from concourse.bass_utils import run_bass_kernel_spmd

    arr = _prep_inputs(**inputs)
    nc = _get_nc()
    in_maps = []
    for k in range(NC):
        m = dict(arr)
        m["selT"] = _make_selT(k)
        in_maps.append(m)
    res = run_bass_kernel_spmd(nc, in_maps, core_ids=list(range(NC)))
    grid = np.concatenate([res.results[k]["grid"] for k in range(NC)], axis=0)
    mask = np.ones((N, N), dtype=bool)
    np.fill_diagonal(mask, False)
    mask[:, 0] = False
    return grid[mask].reshape(-1, 1).astype(np.float32)
